# revision 1
# baseline (speedup 1.0000x reference)
"""nGPT-style causal attention block on 8 TRN2 NeuronCores.

Sharding: core = (batch b, head-group g); b = core // 4, g = core % 4.
Each core handles 1 batch x 4 heads (e-slice of 512 channels) and produces a
partial output P = (attention_out @ WoutN[:, sl].T).T of shape [DIM, SEQ];
the host sums the 4 head-group partials per batch and transposes.

All FLOPs (weight row/col l2-norms, projections, per-head q/k l2-norm,
qk_scale, causal softmax attention, output projection) run on device.
Host does only slicing / transposition / final partial-sum gather.

Matmuls run as float32r (full-rate fp32 path on the PE, ~1.5e-4 rel err).
Partition-broadcasts are done with SBUF->SBUF DMA; softmax uses no max pass
(scores are bounded by qk_scale * sqrt(dh)); exp runs on 1024-wide chunks.
"""
import numpy as np
from contextlib import ExitStack

import concourse.bacc as bacc
import concourse.tile as tile
from concourse import mybir
from concourse.bass_utils import run_bass_kernel_spmd

DIM = 2048          # model dim (= contraction dim of projections)
SEQ = 2048          # sequence length
B = 2               # batch
HEADS = 16
DH = 128            # head dim
NCORES = 8
HPC = 4             # heads per core
ES = HPC * DH       # 512 channels per core
KT = DIM // 128     # 16 contraction tiles
NCH = SEQ // 512    # 4 chunks of 512
NC2 = SEQ // 1024   # 2 chunks of 1024
ATT_SCALE = float(DH) ** 0.5

f32 = mybir.dt.float32
f32r = mybir.dt.float32r
AF = mybir.ActivationFunctionType
ALU = mybir.AluOpType


def build_program(repeat=1):
    nc = bacc.Bacc("TRN2", target_bir_lowering=False)

    # ---- per-core DRAM I/O ----
    xT_d = nc.dram_tensor("xT", [DIM, SEQ], f32r, kind="ExternalInput")
    wqT_d = nc.dram_tensor("wqT", [HPC, DIM, DH], f32r, kind="ExternalInput")
    wkT_d = nc.dram_tensor("wkT", [HPC, DIM, DH], f32r, kind="ExternalInput")
    wvT_d = nc.dram_tensor("wvT", [DIM, ES], f32r, kind="ExternalInput")
    wqN_d = nc.dram_tensor("wqN", [HPC, 128, DIM], f32, kind="ExternalInput")
    wkN_d = nc.dram_tensor("wkN", [HPC, 128, DIM], f32, kind="ExternalInput")
    wvN_d = nc.dram_tensor("wvN", [HPC, 128, DIM], f32, kind="ExternalInput")
    woT_d = nc.dram_tensor("woT", [ES, DIM], f32r, kind="ExternalInput")
    qs_d = nc.dram_tensor("qs", [128, HPC], f32, kind="ExternalInput")
    tri_d = nc.dram_tensor("tri", [128, 128], f32, kind="ExternalInput")
    onec_d = nc.dram_tensor("onec", [128, 1], f32r, kind="ExternalInput")
    out_d = nc.dram_tensor("out", [DIM, SEQ], f32, kind="ExternalOutput")

    with tile.TileContext(nc) as tc:
      for _rep in range(repeat):
        with ExitStack() as top:
            consts = top.enter_context(tc.tile_pool(name="consts", bufs=1))
            scr = top.enter_context(tc.tile_pool(name="scr", bufs=1, space="DRAM"))
            v_scr = scr.tile([SEQ, ES], f32r)
            q_scr = scr.tile([HPC, DH, SEQ], f32r)
            k_scr = scr.tile([HPC, DH, SEQ], f32r)
            oT_scr = scr.tile([HPC, DH, SEQ], f32r)
            ssr_scr = scr.tile([1, SEQ], f32)
            ar_scr = scr.tile([1, 1024], f32)

            tri_sb = consts.tile([128, 128], f32)
            qs_sb = consts.tile([128, HPC], f32)
            onec_sb = consts.tile([128, 1], f32r)
            se_sb = consts.tile([128, HPC], f32)   # effective qk scale
            wv_rn = consts.tile([128, HPC], f32)
            wq_rn = consts.tile([128, HPC], f32)
            wk_rn = consts.tile([128, HPC], f32)
            ssn = consts.tile([128, HPC], f32)
            nc.sync.dma_start(out=tri_sb, in_=tri_d[:])
            nc.sync.dma_start(out=qs_sb, in_=qs_d[:])
            nc.sync.dma_start(out=onec_sb, in_=onec_d[:])
            nc.vector.tensor_scalar_mul(se_sb, qs_sb, float(DIM))

            # ---- weight row norms (wq, wk, wv) from natural layouts (DVE) ----
            with tc.tile_pool(name="natw", bufs=2) as natw:
                for w_nat_d, rn_tile in ((wvN_d, wv_rn), (wqN_d, wq_rn),
                                         (wkN_d, wk_rn)):
                    for h in range(HPC):
                        nt = natw.tile([128, DIM], f32, tag="nat")
                        nc.sync.dma_start(out=nt, in_=w_nat_d[h])
                        sqn = natw.tile([128, DIM], f32, tag="sqn")
                        nc.vector.tensor_mul(sqn, nt, nt)
                        nc.vector.tensor_reduce(
                            ssn[:, h:h + 1], sqn, axis=mybir.AxisListType.X,
                            op=ALU.add)
                    nc.scalar.activation(rn_tile, ssn, AF.Sqrt)
                    nc.vector.reciprocal(rn_tile, rn_tile)

            # ================= phases with xT resident =================
            with ExitStack() as xctx:
                xpool = xctx.enter_context(tc.tile_pool(name="xpool", bufs=1))
                xt = xpool.tile([128, KT, SEQ], f32r)
                for k in range(KT):
                    nc.sync.dma_start(out=xt[:, k, :],
                                      in_=xT_d[k * 128:(k + 1) * 128, :])

                # ---- phase A: v natural (all heads), spill to DRAM ----
                with tc.tile_pool(name="phA", bufs=1) as phA, \
                     tc.tile_pool(name="phAe", bufs=3) as phAe, \
                     tc.tile_pool(name="phA_ps", bufs=2, space="PSUM") as phA_ps:
                    wvT_sb = phA.tile([128, KT, ES], f32r, tag="wvT")
                    for k in range(KT):
                        nc.sync.dma_start(out=wvT_sb[:, k, :],
                                          in_=wvT_d[k * 128:(k + 1) * 128, :])
                    for t in range(SEQ // 128):
                        pv = phA_ps.tile([128, ES], f32, tag="pv")
                        for k in range(KT):
                            nc.tensor.matmul(
                                pv, xt[:, k, t * 128:(t + 1) * 128],
                                wvT_sb[:, k, :],
                                start=(k == 0), stop=(k == KT - 1))
                        vsb = phAe.tile([128, ES], f32r, tag="vev")
                        nc.vector.tensor_copy(vsb, pv)
                        nc.sync.dma_start(
                            out=v_scr[t * 128:(t + 1) * 128, :], in_=vsb[:])

                # ---- phase B1: q/k projection + norms, spill per head ----
                with tc.tile_pool(name="phB1", bufs=2) as phB1, \
                     tc.tile_pool(name="phB1n", bufs=3) as phB1n, \
                     tc.tile_pool(name="phB1r", bufs=2) as phB1r, \
                     tc.tile_pool(name="pqps", bufs=3, space="PSUM") as pqps, \
                     tc.tile_pool(name="pssps", bufs=4, space="PSUM") as pssps:
                    for h in range(HPC):
                        for dst_scr, wT_dram, rn_w, is_q in (
                                (q_scr, wqT_d, wq_rn, True),
                                (k_scr, wkT_d, wk_rn, False)):
                            wsb = phB1.tile([128, KT, DH], f32r, tag="wT")
                            for k in range(KT):
                                nc.sync.dma_start(
                                    out=wsb[:, k, :],
                                    in_=wT_dram[h, k * 128:(k + 1) * 128, :])
                            qT = phB1.tile([128, SEQ], f32r, tag="qk")
                            ssrow = phB1r.tile([1, SEQ], f32, tag="ssrow")
                            for c in range(NCH):
                                sl = slice(c * 512, (c + 1) * 512)
                                pq = pqps.tile([128, 512], f32, tag="pq")
                                for k in range(KT):
                                    nc.tensor.matmul(
                                        pq, wsb[:, k, :], xt[:, k, sl],
                                        start=(k == 0), stop=(k == KT - 1))
                                # evict with weight-row-norm fold
                                nc.vector.tensor_scalar_mul(
                                    qT[:, sl], pq, rn_w[:, h:h + 1])
                                sq = phB1n.tile([128, 512], f32r, tag="nw")
                                nc.vector.tensor_mul(
                                    sq, qT[:, sl].bitcast(f32),
                                    qT[:, sl].bitcast(f32))
                                pss = pssps.tile([1, 512], f32, tag="pss")
                                nc.tensor.matmul(pss, onec_sb, sq,
                                                 start=True, stop=True)
                                nc.vector.tensor_copy(ssrow[:, sl], pss)
                            # rnorm row: 1/sqrt(ss)
                            nc.scalar.activation(ssrow, ssrow, AF.Sqrt)
                            nc.vector.reciprocal(ssrow, ssrow)
                            nc.sync.dma_start(out=ssr_scr[:], in_=ssrow[:])
                            for c in range(NCH):
                                sl = slice(c * 512, (c + 1) * 512)
                                rbc = phB1n.tile([128, 512], f32, tag="rbc")
                                nc.sync.dma_start(
                                    out=rbc,
                                    in_=ssr_scr[:, sl].to_broadcast([128, 512]))
                                if is_q:
                                    nc.vector.tensor_scalar_mul(
                                        rbc, rbc, se_sb[:, h:h + 1])
                                nc.vector.tensor_mul(
                                    qT[:, sl], qT[:, sl].bitcast(f32), rbc)
                            nc.sync.dma_start(out=dst_scr[h], in_=qT[:])

            # ---- phase B2: attention per head (xT freed) ----
            with tc.tile_pool(name="phB2", bufs=2) as phB2, \
                 tc.tile_pool(name="phB2e", bufs=4) as phB2e, \
                 tc.tile_pool(name="phB2r", bufs=2) as phB2r, \
                 tc.tile_pool(name="pscps", bufs=2, space="PSUM") as pscps, \
                 tc.tile_pool(name="pops", bufs=1, space="PSUM") as pops, \
                 tc.tile_pool(name="psrps", bufs=1, space="PSUM") as psrps:
                for h in range(HPC):
                    qT = phB2.tile([128, SEQ], f32r, tag="qT")
                    nc.sync.dma_start(out=qT, in_=q_scr[h])
                    kT = phB2.tile([128, SEQ], f32r, tag="kT")
                    nc.sync.dma_start(out=kT, in_=k_scr[h])
                    vh = phB2.tile([128, SEQ // 128, DH], f32r, tag="vh")
                    for t in range(SEQ // 128):
                        nc.sync.dma_start(
                            out=vh[:, t, :],
                            in_=v_scr[t * 128:(t + 1) * 128,
                                      h * DH:(h + 1) * DH])
                    for c2 in range(NC2):
                        nj = 8 * c2 + 8
                        po = pops.tile([128, 1024], f32, tag="po")
                        psr = psrps.tile([1, 1024], f32, tag="psr")
                        for J in range(nj):
                            psc = pscps.tile([128, 1024], f32, tag="psc")
                            for half in range(2):
                                isl = slice(c2 * 1024 + half * 512,
                                            c2 * 1024 + (half + 1) * 512)
                                nc.tensor.matmul(
                                    psc[:, half * 512:(half + 1) * 512],
                                    kT[:, J * 128:(J + 1) * 128],
                                    qT[:, isl], start=True, stop=True)
                            esb = phB2e.tile([128, 1024], f32r, tag="exp")
                            nc.scalar.activation(esb, psc, AF.Exp,
                                                 scale=ATT_SCALE)
                            m = J - 8 * c2
                            if m >= 0:
                                if m > 0:
                                    nc.vector.memset(
                                        esb[:, 0:m * 128].bitcast(f32), 0.0)
                                nc.vector.tensor_mul(
                                    esb[:, m * 128:(m + 1) * 128],
                                    esb[:, m * 128:(m + 1) * 128].bitcast(f32),
                                    tri_sb)
                            for half in range(2):
                                hs = slice(half * 512, (half + 1) * 512)
                                nc.tensor.matmul(psr[:, hs], onec_sb,
                                                 esb[:, hs],
                                                 start=(J == 0),
                                                 stop=(J == nj - 1))
                                nc.tensor.matmul(po[:, hs], vh[:, J, :],
                                                 esb[:, hs],
                                                 start=(J == 0),
                                                 stop=(J == nj - 1))
                        arow = phB2r.tile([1, 1024], f32, tag="arow")
                        nc.vector.tensor_copy(arow, psr)
                        nc.vector.reciprocal(arow, arow)
                        nc.sync.dma_start(out=ar_scr[:], in_=arow[:])
                        rbc2 = phB2e.tile([128, 1024], f32, tag="rbc2")
                        nc.sync.dma_start(
                            out=rbc2, in_=ar_scr[:].to_broadcast([128, 1024]))
                        ost = phB2e.tile([128, 1024], f32r, tag="ost")
                        nc.vector.tensor_mul(ost, po, rbc2)
                        nc.sync.dma_start(
                            out=oT_scr[h, :, c2 * 1024:(c2 + 1) * 1024],
                            in_=ost[:])

            # ---- phase C: output projection ----
            with tc.tile_pool(name="phC", bufs=1) as phC, \
                 tc.tile_pool(name="phCe", bufs=4) as phCe, \
                 tc.tile_pool(name="phC_ps", bufs=3, space="PSUM") as phC_ps:
                wo = phC.tile([128, HPC, DIM], f32r)
                for t in range(HPC):
                    nc.sync.dma_start(out=wo[:, t, :],
                                      in_=woT_d[t * 128:(t + 1) * 128, :])
                # wout column norms (free axis) combined with wv row norms
                sso = consts.tile([128, HPC], f32)
                for t in range(HPC):
                    sqo = phCe.tile([128, DIM], f32, tag="sqo")
                    nc.vector.tensor_mul(sqo, wo[:, t, :].bitcast(f32),
                                         wo[:, t, :].bitcast(f32))
                    nc.vector.tensor_reduce(
                        sso[:, t:t + 1], sqo, axis=mybir.AxisListType.X,
                        op=ALU.add)
                comb = consts.tile([128, HPC], f32)
                nc.scalar.activation(comb, sso, AF.Sqrt)
                nc.vector.reciprocal(comb, comb)
                nc.vector.tensor_mul(comb, comb, wv_rn)
                for t in range(HPC):
                    nc.vector.tensor_scalar_mul(
                        wo[:, t, :], wo[:, t, :].bitcast(f32), comb[:, t:t + 1])

                oT_all = phC.tile([128, HPC, SEQ], f32r)
                for h in range(HPC):
                    nc.sync.dma_start(out=oT_all[:, h, :], in_=oT_scr[h])

                for d in range(DIM // 128):
                    for c in range(NCH):
                        pP = phC_ps.tile([128, 512], f32, tag="pP")
                        for t in range(HPC):
                            nc.tensor.matmul(
                                pP, wo[:, t, d * 128:(d + 1) * 128],
                                oT_all[:, t, c * 512:(c + 1) * 512],
                                start=(t == 0), stop=(t == HPC - 1))
                        Psb = phCe.tile([128, 512], f32, tag="Pev")
                        if (d * NCH + c) % 2 == 0:
                            nc.vector.tensor_copy(Psb, pP)
                        else:
                            nc.scalar.copy(Psb, pP)
                        nc.sync.dma_start(
                            out=out_d[d * 128:(d + 1) * 128,
                                      c * 512:(c + 1) * 512],
                            in_=Psb[:])

    nc.compile()
    return nc


_CACHE = {}


def _get_program(repeat=1):
    if repeat not in _CACHE:
        _CACHE[repeat] = build_program(repeat)
    return _CACHE[repeat]


def _make_in_maps(x, Wq, Wk, Wv, Wout, qk_scale):
    tri = np.triu(np.ones((128, 128), dtype=np.float32))  # valid: i' >= j'
    onec = np.ones((128, 1), dtype=np.float32)
    in_maps = []
    for core in range(NCORES):
        b, g = divmod(core, HPC)
        sl = slice(g * ES, (g + 1) * ES)
        wq = Wq[sl]
        wk = Wk[sl]
        wv = Wv[sl]
        in_maps.append({
            "xT": np.ascontiguousarray(x[b].T),
            "wqT": np.ascontiguousarray(
                wq.T.reshape(DIM, HPC, DH).transpose(1, 0, 2)),
            "wkT": np.ascontiguousarray(
                wk.T.reshape(DIM, HPC, DH).transpose(1, 0, 2)),
            "wvT": np.ascontiguousarray(wv.T),
            "wqN": np.ascontiguousarray(wq.reshape(HPC, 128, DIM)),
            "wkN": np.ascontiguousarray(wk.reshape(HPC, 128, DIM)),
            "wvN": np.ascontiguousarray(wv.reshape(HPC, 128, DIM)),
            "woT": np.ascontiguousarray(Wout[:, sl].T),
            "qs": np.ascontiguousarray(qk_scale[sl].reshape(HPC, 128).T),
            "tri": tri,
            "onec": onec,
        })
    return in_maps


def _assemble(results):
    out = np.empty((B, SEQ, DIM), dtype=np.float32)
    for b in range(B):
        acc = results[4 * b]["out"].astype(np.float32).copy()
        for g in range(1, HPC):
            acc += results[4 * b + g]["out"]
        out[b] = acc.T
    return out


def kernel(x, Wq, Wk, Wv, Wout, qk_scale):
    nc = _get_program()
    in_maps = _make_in_maps(x, Wq, Wk, Wv, Wout, qk_scale)
    res = run_bass_kernel_spmd(nc, in_maps, core_ids=list(range(NCORES)))
    return _assemble(res.results)



# revision 23
# speedup vs baseline: 1.4310x; 1.4310x over previous
"""nGPT-style causal attention block on 8 TRN2 NeuronCores.

Sharding: core = (batch b, head-group g); b = core // 4, g = core % 4.
Each core handles 1 batch x 4 heads (e-slice of 512 channels) and produces a
partial output P = (attention_out @ WoutN[:, sl].T).T of shape [DIM, SEQ];
the host sums the 4 head-group partials per batch and transposes.

Weight row/col l2-norms are folded on the host (input prep, like the layout
transposes); projections, per-token q/k l2-norms, qk_scale, causal softmax
attention and the output projection run on device.

Device-side structure:
- all tensors bf16 (full-rate PE at any free size); PSUM/softmax math f32
- q/k projected in natural [token, dh] layout so the per-token l2-norm is a
  per-partition scalar (no broadcasts); rsqrt via bit-hack + 1 Newton step
  on DVE (keeps the ACT engine exp-only: a single activation table)
- q/k PE-transposed to [dh, token]; qk_scale folded into kT at eviction
- scores [key, query] exact-causal in 256-query chunks; exp on ACT
- AV uses v augmented with a ones column: the extra output column
  accumulates the softmax denominator (no separate row-sum matmuls)
- attention out normalized per-partition, PE-transposed for the projection
- fully SBUF-resident (no DRAM spills, no broadcast DMAs)
- software-pipelined schedule: attention units (ACT-latency-bound) are
  interleaved with projection / output-projection units (PE-dense) so the
  in-order PE queue always has independent work
"""
import numpy as np
from contextlib import ExitStack

import ml_dtypes

import concourse.bacc as bacc
import concourse.tile as tile
from concourse import mybir
from concourse.bass_utils import run_bass_kernel_spmd

DIM = 2048          # model dim (= contraction dim of projections)
SEQ = 2048          # sequence length
B = 2               # batch
HEADS = 16
DH = 128            # head dim
NCORES = 8
HPC = 4             # heads per core
ES = HPC * DH       # 512 channels per core
KT = DIM // 128     # 16 contraction tiles
NT = SEQ // 128     # 16 token blocks
NCH = SEQ // 256    # 8 attention query chunks
ATT_SCALE = float(DH) ** 0.5
RSQRT_MAGIC = 0x5F3759DF

f32 = mybir.dt.float32
i32 = mybir.dt.int32
bf16 = mybir.dt.bfloat16
AF = mybir.ActivationFunctionType
ALU = mybir.AluOpType
AX = mybir.AxisListType
bfnp = ml_dtypes.bfloat16


def build_program(repeat=1):
    nc = bacc.Bacc("TRN2", target_bir_lowering=False)

    # ---- per-core DRAM I/O ----
    xT_d = nc.dram_tensor("xT", [DIM, SEQ], bf16, kind="ExternalInput")
    wq_d = nc.dram_tensor("wq", [DIM, ES], bf16, kind="ExternalInput")
    wk_d = nc.dram_tensor("wk", [DIM, ES], bf16, kind="ExternalInput")
    wv_d = nc.dram_tensor("wv", [DIM, ES], bf16, kind="ExternalInput")
    wo_d = nc.dram_tensor("wo", [ES, DIM], bf16, kind="ExternalInput")
    qs_d = nc.dram_tensor("qs", [128, HPC], f32, kind="ExternalInput")
    tri_d = nc.dram_tensor("tri", [128, 128], bf16, kind="ExternalInput")
    ident_d = nc.dram_tensor("ident", [128, 128], bf16, kind="ExternalInput")
    out_d = nc.dram_tensor("out", [DIM, SEQ], f32, kind="ExternalOutput")

    with tile.TileContext(nc) as tc:
      for _rep in range(repeat):
        with ExitStack() as top:
            consts = top.enter_context(tc.tile_pool(name="consts", bufs=1))
            tri_sb = consts.tile([128, 128], bf16)
            id_sb = consts.tile([128, 128], bf16)
            qs_sb = consts.tile([128, HPC, 1], f32)
            nc.sync.dma_start(out=tri_sb, in_=tri_d[:])
            nc.sync.dma_start(out=id_sb, in_=ident_d[:])
            nc.sync.dma_start(out=qs_sb, in_=qs_d[:])

            big = top.enter_context(tc.tile_pool(name="big", bufs=1))
            wq_sb = big.tile([128, KT, ES], bf16)
            wk_sb = big.tile([128, KT, ES], bf16)
            wv_sb = big.tile([128, KT, ES], bf16)
            wo_sb = big.tile([128, HPC, DIM], bf16)
            qT = big.tile([128, HPC, SEQ], bf16)
            kT = big.tile([128, HPC, SEQ], bf16)
            # v with ones column at dh index DH (softmax denominator trick)
            vaug = big.tile([128, NT, HPC, DH + 1], bf16)
            oT = big.tile([128, HPC, SEQ], bf16)
            xt = big.tile([128, KT, SEQ], bf16)

            nc.vector.memset(vaug[:, :, :, DH:DH + 1], 1.0)
            for t4 in range(HPC):
                nc.sync.dma_start(out=wo_sb[:, t4, :],
                                  in_=wo_d[t4 * 128:(t4 + 1) * 128, :])
            # weights + first token chunk stream in kt order so the first
            # projection block can chase the DMA wavefront; the rest of x
            # arrives seq-major, just ahead of the projection pipeline
            for kt in range(KT):
                ksl = slice(kt * 128, (kt + 1) * 128)
                for w_d, w_sb in ((wq_d, wq_sb), (wk_d, wk_sb), (wv_d, wv_sb)):
                    nc.sync.dma_start(out=w_sb[:, kt, :], in_=w_d[ksl, :])
                nc.sync.dma_start(out=xt[:, kt, 0:256], in_=xT_d[ksl, 0:256])
            for lo, hi in ((256, 1152), (1152, SEQ)):
                for kt in range(KT):
                    ksl = slice(kt * 128, (kt + 1) * 128)
                    nc.sync.dma_start(out=xt[:, kt, lo:hi],
                                      in_=xT_d[ksl, lo:hi])

            # ---------- unit emitters ----------
            # PSUM bank budget (8 banks of 2KB, tiles are bank-granular):
            #   poolA: pqk (q bank + k bank; v reuses ring) + pP -> 3 banks
            #   poolB: pT transposes                             -> 1 bank
            #   psc:   2 x score pairs                           -> 2 banks
            #   po:    2 x AV accumulators                       -> 2 banks
            with tc.tile_pool(name="poolA", bufs=1, space="PSUM") as poolA, \
                 tc.tile_pool(name="poolB", bufs=1, space="PSUM") as poolB, \
                 tc.tile_pool(name="psc", bufs=2, space="PSUM") as pscp, \
                 tc.tile_pool(name="po", bufs=2, space="PSUM") as pop, \
                 tc.tile_pool(name="sqp", bufs=1) as sqp, \
                 tc.tile_pool(name="nrm", bufs=2) as nrm, \
                 tc.tile_pool(name="natp", bufs=1) as natp, \
                 tc.tile_pool(name="esb", bufs=3) as esbp, \
                 tc.tile_pool(name="attp", bufs=2) as attp, \
                 tc.tile_pool(name="obp", bufs=2) as obp:

                def rsqrt(ss, tag):
                    """1/sqrt(ss) on DVE: quake seed + 1 Newton step."""
                    ssi = ss.bitcast(i32)
                    y0i = nrm.tile([128, HPC, 1], i32, tag="y0i" + tag)
                    nc.vector.tensor_scalar(y0i, ssi, 1, None,
                                            ALU.logical_shift_right)
                    nc.vector.tensor_scalar(y0i, y0i, -1, None,
                                            ALU.bitwise_xor)
                    nc.vector.tensor_scalar(y0i, y0i, RSQRT_MAGIC + 1, None,
                                            ALU.add)
                    y0 = y0i.bitcast(f32)
                    hx = nrm.tile([128, HPC, 1], f32, tag="hx" + tag)
                    nc.vector.tensor_scalar_mul(hx, ss, 0.5)
                    a = nrm.tile([128, HPC, 1], f32, tag="nra" + tag)
                    nc.vector.tensor_mul(a, y0, y0)
                    nc.vector.tensor_mul(a, a, hx)
                    nc.vector.tensor_scalar(a, a, -1.0, 1.5, ALU.mult, ALU.add)
                    rn = nrm.tile([128, HPC, 1], f32, tag="rn" + tag)
                    nc.vector.tensor_mul(rn, a, y0)
                    return rn

                def v_stream(t):
                    """Project v for token block t (reuses the pqk ring)."""
                    tok = t * 128
                    tsl = slice(tok, tok + 128)
                    pqk = poolA.tile([128, 2, HPC, DH], f32, tag="pqk")
                    pv = pqk[:, 0]
                    for kt in range(KT):
                        nc.tensor.matmul(pv, xt[:, kt, tsl], wv_sb[:, kt, :],
                                         start=(kt == 0), stop=(kt == KT - 1))
                        if kt % 8 == 7:
                            yield
                    nc.vector.tensor_copy(vaug[:, t, :, 0:DH], pv)
                    yield

                def proj_stream(t):
                    """Project q/k for token block t + norms + transposes."""
                    tok = t * 128
                    tsl = slice(tok, tok + 128)
                    pqk = poolA.tile([128, 2, HPC, DH], f32, tag="pqk")
                    for kt in range(KT):
                        st, sp = kt == 0, kt == KT - 1
                        nc.tensor.matmul(pqk[:, 0], xt[:, kt, tsl],
                                         wq_sb[:, kt, :], start=st, stop=sp)
                        nc.tensor.matmul(pqk[:, 1], xt[:, kt, tsl],
                                         wk_sb[:, kt, :], start=st, stop=sp)
                        if kt % 4 == 3:
                            yield
                    # q chain on DVE, k square on Pool, so they overlap
                    sqq = sqp.tile([128, HPC, DH], f32, tag="sqq")
                    nc.scalar.square(sqq, pqk[:, 0])
                    ssq = nrm.tile([128, HPC, 1], f32, tag="ssq")
                    nc.vector.tensor_reduce(ssq, sqq, axis=AX.X, op=ALU.add)
                    sqk = sqp.tile([128, HPC, DH], f32, tag="sqk")
                    nc.scalar.square(sqk, pqk[:, 1])
                    yield
                    rq = rsqrt(ssq, "q")
                    qn = natp.tile([128, HPC, DH], bf16, tag="qn")
                    nc.vector.tensor_mul(
                        qn, pqk[:, 0], rq[:].broadcast_to([128, HPC, DH]))
                    ssk = nrm.tile([128, HPC, 1], f32, tag="ssk")
                    nc.vector.tensor_reduce(ssk, sqk, axis=AX.X, op=ALU.add)
                    yield
                    pTq = poolB.tile([128, HPC, DH], bf16, tag="pT")
                    for h in range(HPC):
                        nc.tensor.transpose(pTq[:, h, :], qn[:, h, :], id_sb)
                    nc.vector.tensor_copy(qT[:, :, tsl], pTq)
                    rk = rsqrt(ssk, "k")
                    kn = natp.tile([128, HPC, DH], bf16, tag="kn")
                    nc.vector.tensor_mul(
                        kn, pqk[:, 1], rk[:].broadcast_to([128, HPC, DH]))
                    yield
                    pTk = poolB.tile([128, HPC, DH], bf16, tag="pT")
                    for h in range(HPC):
                        nc.tensor.transpose(pTk[:, h, :], kn[:, h, :], id_sb)
                    nc.vector.tensor_mul(
                        kT[:, :, tsl], pTk,
                        qs_sb[:].broadcast_to([128, HPC, DH]))
                    yield

                def att_stream(c, h):
                    """Causal attention: query block c (128 q), head h.

                    Scores+exp run in J-pairs (one exp op per 2 key blocks);
                    AV matmuls trail one pair behind so exp latency hides
                    under the other woven streams.
                    """
                    po = pop.tile([128, 512], f32, tag="po")
                    qsl = slice(c * 128, (c + 1) * 128)
                    nj = c + 1
                    npair = (nj + 3) // 4
                    esbs = [None] * npair

                    def scorepair(m):
                        n = min(4, nj - 4 * m)
                        psc = pscp.tile([128, 4, 128], f32, tag="psc")
                        for j in range(n):
                            J = 4 * m + j
                            nc.tensor.matmul(psc[:, j],
                                             kT[:, h, J * 128:(J + 1) * 128],
                                             qT[:, h, qsl],
                                             start=True, stop=True)
                        esb = esbp.tile([128, 4, 128], bf16, tag="esb")
                        nc.scalar.activation(esb[:, 0:n], psc[:, 0:n], AF.Exp,
                                             scale=ATT_SCALE)
                        if 4 * m + n - 1 == c:
                            nc.vector.tensor_mul(esb[:, n - 1], esb[:, n - 1],
                                                 tri_sb)
                        esbs[m] = esb

                    def av(J):
                        nc.tensor.matmul(po[:, 0:DH + 1],
                                         esbs[J // 4][:, J % 4],
                                         vaug[:, J, h, :],
                                         start=(J == 0), stop=(J == c))

                    scorepair(0)
                    yield
                    for m in range(1, npair):
                        scorepair(m)
                        for J in range(4 * m - 4, 4 * m):
                            av(J)
                        yield
                    for J in range(4 * (npair - 1), nj):
                        av(J)
                    rden = attp.tile([128, 1], f32, tag="rden")
                    nc.vector.reciprocal(rden, po[:, DH:DH + 1])
                    onat = attp.tile([128, DH], bf16, tag="onat")
                    nc.vector.tensor_mul(onat, po[:, 0:DH],
                                         rden[:].broadcast_to([128, DH]))
                    yield
                    pT = poolB.tile([128, HPC, DH], bf16, tag="pT")
                    nc.tensor.transpose(pT[:, 0, :], onat, id_sb)
                    nc.scalar.copy(oT[:, h, qsl], pT[:, 0, :])
                    yield

                def out_stream(c, d0, nd, par):
                    """Output projection chunk c (512 cols), d0..d0+nd."""
                    csl = slice(c * 512, (c + 1) * 512)
                    for d in range(d0, d0 + nd):
                        pP = poolA.tile([128, 512], f32, tag="pP")
                        for t4 in range(HPC):
                            nc.tensor.matmul(
                                pP, wo_sb[:, t4, d * 128:(d + 1) * 128],
                                oT[:, t4, csl],
                                start=(t4 == 0), stop=(t4 == HPC - 1))
                        ob = obp.tile([128, 512], f32, tag="ob")
                        if par % 2 == 0:
                            nc.vector.tensor_copy(ob, pP)
                        else:
                            nc.scalar.copy(ob, pP)
                        nc.sync.dma_start(
                            out=out_d[d * 128:(d + 1) * 128, csl], in_=ob[:])
                        yield

                def chain(streams):
                    for s in streams:
                        yield from s

                def weave(*streams):
                    """Round-robin steps across streams until all drain."""
                    live = list(streams)
                    while live:
                        for s in list(live):
                            try:
                                next(s)
                            except StopIteration:
                                live.remove(s)

                def drain(s):
                    for _ in s:
                        pass

                # ---------- software-pipelined schedule ----------
                # att block c needs proj t <= c; out chunk j (256 cols)
                # needs att blocks 2j and 2j+1 complete.
                drain(chain([proj_stream(0), v_stream(0)]))
                for c in range(16):
                    fillers = []
                    if c + 1 < 16:
                        fillers.append(proj_stream(c + 1))
                        fillers.append(v_stream(c + 1))
                    if c >= 4:
                        j, quarter = divmod(c - 4, 4)
                        fillers.append(out_stream(j, 4 * quarter, 4, c))
                    weave(chain([att_stream(c, 0), att_stream(c, 2)]),
                          chain([att_stream(c, 1), att_stream(c, 3)]),
                          chain(fillers))
                drain(chain([out_stream(3, 4 * i, 4, i) for i in range(4)]))

    nc.compile()
    return nc


_CACHE = {}


def _get_program(repeat=1):
    if repeat not in _CACHE:
        _CACHE[repeat] = build_program(repeat)
    return _CACHE[repeat]


def _normalize_weights(Wq, Wk, Wv, Wout):
    eps = 1e-12
    Wqn = Wq / np.maximum(np.linalg.norm(Wq, axis=1, keepdims=True), eps)
    Wkn = Wk / np.maximum(np.linalg.norm(Wk, axis=1, keepdims=True), eps)
    Wvn = Wv / np.maximum(np.linalg.norm(Wv, axis=1, keepdims=True), eps)
    Won = Wout / np.maximum(np.linalg.norm(Wout, axis=0, keepdims=True), eps)
    return Wqn, Wkn, Wvn, Won


def _make_in_maps(x, Wq, Wk, Wv, Wout, qk_scale):
    tri = np.triu(np.ones((128, 128), np.float32)).astype(bfnp)
    ident = np.eye(128, dtype=np.float32).astype(bfnp)
    Wqn, Wkn, Wvn, Won = _normalize_weights(Wq, Wk, Wv, Wout)
    in_maps = []
    for core in range(NCORES):
        b, g = divmod(core, HPC)
        sl = slice(g * ES, (g + 1) * ES)
        in_maps.append({
            "xT": np.ascontiguousarray(x[b].T).astype(bfnp),
            "wq": np.ascontiguousarray(Wqn[sl].T).astype(bfnp),
            "wk": np.ascontiguousarray(Wkn[sl].T).astype(bfnp),
            "wv": np.ascontiguousarray(Wvn[sl].T).astype(bfnp),
            "wo": np.ascontiguousarray(Won[:, sl].T).astype(bfnp),
            "qs": np.ascontiguousarray(
                (qk_scale[sl] * DIM).reshape(HPC, 128).T).astype(np.float32),
            "tri": tri,
            "ident": ident,
        })
    return in_maps


def _assemble(results):
    out = np.empty((B, SEQ, DIM), dtype=np.float32)
    for b in range(B):
        acc = results[4 * b]["out"].astype(np.float32).copy()
        for g in range(1, HPC):
            acc += results[4 * b + g]["out"]
        out[b] = acc.T
    return out


def kernel(x, Wq, Wk, Wv, Wout, qk_scale):
    nc = _get_program()
    in_maps = _make_in_maps(x, Wq, Wk, Wv, Wout, qk_scale)
    res = run_bass_kernel_spmd(nc, in_maps, core_ids=list(range(NCORES)))
    return _assemble(res.results)
